# revision 77
# baseline (speedup 1.0000x reference)
"""AdvisorCrossAttentionAdapter Trainium2 kernel.

Full inputs in, full outputs out. Sharding: 8 cores = 4 batches x 2 query
halves; the pair sharing a batch also tensor-parallels (by output columns)
the per-batch shared projections, exchanging halves with small pair
AllGathers that hide behind compute.

Math notes (per batch):
  - K projection is folded into the keys: kM = a0 @ M^T with M = Wq^T Wk
    precomputed on the host, so scores = hidden @ kM^T. Folding into K
    (T=1024 rows) instead of Q (S=2048 rows) halves the fold cost.
  - Wo is folded through the value path: with the id-gate rewritten as a
    linear part plus one sparse abs-term (host-compacted, padded to cpad),
        v_final = adv_lin @ Wv^T + Pc @ |sc @ Wv^T|
    the attention output becomes out = attn_norm @ vo, where
        vo = adv_lin @ WVO + Pc @ (|sc @ Wv^T| @ Wo^T),  WVO = Wv^T Wo^T
    (WVO precomputed on the host). The separate ctx and out-projection
    phases collapse into one attn @ vo matmul.
  - Per-core work: abscT 2.15 + kM 6.4 (half TP'd, half duplicated) +
    acWo 2.15 + vo-lin 4.3 + scatter 1.1 + scores 4.3 + out 4.3 =
    24.7 GFLOP (vs 33.3 for the qm-side variant).
  - TP splits are all by OUTPUT COLUMNS (abscT rows = Wv output cols, kM^T
    rows, vo cols), so the pair exchanges are AllGathers (1-2 MB out), not
    the 4 MB AllReduce that sank the earlier vo-partial variant.
  - Softmax runs without max subtraction (scores/sqrt(h) ~ N(0,1)); exp'd
    scores stay unnormalized through the out matmul and the 1/sum factor
    is applied on the HOST (the exp-sums ship as a tiny second output).
  - All matmuls take bf16 inputs with fp32 PSUM accumulation; the output
    is written bf16 and cast to fp32 on the host (~0.2% extra RMS, budget
    is 2e-2).

Schedule notes (the cost model this was tuned against):
  - Collectives serialize on ONE resource at 15us + out_bytes/40GBps each;
    the chain absc -> kM -> vo0 -> vo1 (4 x 67.4us, back to back from
    ~46us) IS the critical path, so: the absc AllGather launches straight
    off phase A; kM is only half-TP'd (the other half's duplicated compute
    is cheaper than a bigger collective and covers the absc window); the
    acWo and vo phases interleave per 512-col block so the first vo
    AllGather fires mid-phase; phases S and the first half of O run under
    the vo collectives; O's o-tiles run in order 0,2,1,3 so only its last
    two tiles wait on the second vo AllGather.
  - All DMA shares ONE serial ~358GB/s resource; a DMA trigger occupies
    its engine's sequencer until dispatched, and the bass scheduler may
    reorder same-engine triggers, so collective-dependent gathers live on
    the otherwise-idle Pool (SWDGE) queue, input streams split between the
    SP/ACT rings in consumption order, and phase A's stationary+rhs
    interleave finely at the front.
"""

import numpy as np
import ml_dtypes
from contextlib import ExitStack

P = 128
H = 2048          # hidden dim
HC = H // P       # 16 h-chunks of 128
T = 1024          # triplets per batch (advisor len 3072 / 3)
TC = T // P       # 8 t-chunks
S = 1024          # query rows per core (2048 / 2)
B = 4
NCORES = 8
CPAD = 512        # padded compact rows (impl + and/or/xor, disjoint)
SCALE = 1.0 / float(np.sqrt(H))

bf16 = ml_dtypes.bfloat16

_compiled_nc = None


def _build_nc4(s_rows=S, t_trip=T, h=H, cpad=CPAD, n_dev=NCORES,
               stop_after=None):
    import concourse.bass as bass
    import concourse.mybir as mybir
    import concourse.tile as tile
    from concourse import bacc

    hc = h // P          # 16 h-chunks
    tc_n = t_trip // P   # 8 t-chunks
    sc_n = s_rows // P   # 8 s-chunks
    s512 = s_rows // 512 # 2
    hv = h // 2          # own half of output columns
    hb = hv // 512       # 2 own 512-blocks
    cpc = cpad // P      # compact-row chunks
    cb_n = cpad // 512   # compact 512-tiles
    n512 = h // 512      # 4 full-width o tiles (phase O)
    assert s_rows % 512 == 0 and h % 1024 == 0 and cpad % 512 == 0

    f32 = mybir.dt.float32
    bf = mybir.dt.bfloat16

    nc = bacc.Bacc("TRN2", target_bir_lowering=False, debug=False,
                   num_devices=n_dev)

    # DRAM I/O. All weight streams are pre-blocked on the host into
    # [block, 128, hc, 512] so each streamed tile is contiguous per
    # partition. "own" = this core's half of the respective output columns
    # (selected purely by the data the host feeds it; the program is SPMD).
    d_sc = nc.dram_tensor("scT", [P, hc, cpad], bf, kind="ExternalInput")
    d_wv = nc.dram_tensor("wvT", [hb, P, hc, 512], bf, kind="ExternalInput")
    # kM rows: the first 1024 (2 x 512-blocks) are tensor-parallel across
    # the pair (mTp = own 512 columns of M^T), the last 1024 duplicated.
    # (TP'ing fewer rows shrinks the kM AllGather but grows the duplicated
    # K compute, which delays phase VL and the vo AllGathers: measured
    # worse.)
    d_mtp = nc.dram_tensor("mTp", [P, hc, 512], bf, kind="ExternalInput")
    d_mt = nc.dram_tensor("mT", [2, P, hc, 512], bf, kind="ExternalInput")
    d_a0 = nc.dram_tensor("a0T", [P, hc, t_trip], bf, kind="ExternalInput")
    d_al = nc.dram_tensor("alT", [t_trip // P, P, hc, P], bf,
                          kind="ExternalInput")
    d_pc = nc.dram_tensor("pcT", [P, cpc, t_trip], bf, kind="ExternalInput")
    d_wvo = nc.dram_tensor("wvoT", [hb, P, hc, 512], bf,
                           kind="ExternalInput")
    d_wo = nc.dram_tensor("woT", [hb, P, hc, 512], bf, kind="ExternalInput")
    d_h = nc.dram_tensor("hT", [P, hc, s_rows], bf, kind="ExternalInput")
    d_out = nc.dram_tensor("out", [s_rows, h], bf,
                            kind="ExternalOutput")
    d_sums = nc.dram_tensor("sums", [1, s_rows], f32,
                            kind="ExternalOutput")

    AF = mybir.ActivationFunctionType

    with tile.TileContext(nc) as tc, ExitStack() as ctx:
        big = ctx.enter_context(tc.tile_pool(name="big", bufs=1))
        pw = ctx.enter_context(tc.tile_pool(name="pw", bufs=2))
        psd = ctx.enter_context(tc.tile_pool(name="psd", bufs=2))
        pgs = ctx.enter_context(tc.tile_pool(name="pgs", bufs=2))
        psm = ctx.enter_context(tc.tile_pool(name="psm", bufs=1))
        pp = ctx.enter_context(tc.tile_pool(name="pp", bufs=6, space="PSUM"))
        ppe = ctx.enter_context(tc.tile_pool(name="ppe", bufs=1,
                                             space="PSUM"))
        dram = ctx.enter_context(tc.tile_pool(name="dram", bufs=1,
                                              space="DRAM"))

        # Persistent intermediates. Tag reuse: a0T -> vo (phase K ends
        # before the vo gather lands), scT -> hT (phase A ends before the
        # hT stream is issued on the weight queue).
        a0T = big.tile([P, hc, t_trip], bf, tag="A", name="a0T")
        kMT = big.tile([P, hc, t_trip], bf, tag="C", name="kMT")
        scT = big.tile([P, hc, max(cpad, 1024)], bf, tag="D", name="scT")
        abscT = big.tile([P, hc, cpad], bf, tag="E", name="abscT")
        pc_sb = psm.tile([P, cpc, t_trip], bf, tag="pc", name="pc_sb")
        acWo = psm.tile([P, cpc, hv], bf, tag="aw", name="acWo")

        # ACT-written zero bias vector so Abs/Exp activations don't pull in
        # a DMA'd const AP (also absorbs the pipeline-RAW wait). Sourced
        # from scT, the first tile to land on the sync queue.
        zbias = psm.tile([P, 1], f32, tag="zb", name="zbias")
        warm = psm.tile([P, 1], f32, tag="wm", name="warm")

        # PE warm-up while the first weight tiles land.
        dummy = psm.tile([P, 512], bf, tag="dm", name="dummy")
        nc.vector.memset(dummy[:], 0.0)
        for _ in range(10):
            ps_dm = pp.tile([P, 512], f32, tag="PS", name="ps_dm")
            nc.tensor.matmul(ps_dm[:], dummy[:, 0:P], dummy[:],
                             start=True, stop=True)

        # Input streams. The sim models ONE serial DMA resource, so global
        # order matters: phase A's stationary (wv block 0) and rhs (scT)
        # stream first, finely interleaved so the first groups ramp with
        # the DMA; everything phase A doesn't need follows on the ACT ring.

        # ---- Phase A: abscT_own[o, c] = |sum_j WvT[j,o] scT[j,c]| --------
        # o = own half of Wv output columns (8 chunks). Staged to DRAM and
        # pair-AllGathered into the full abscT [h, cpad]. (A 2-way split
        # of this collective measured WORSE: ~7us of event/SEQ latency per
        # collective eats the earlier launch.)
        agA_in = dram.tile([hv, cpad], bf, name="agA_in", uniquify=False)
        agA_out = dram.tile([2, hv, cpad], bf, name="agA_out",
                            uniquify=False)
        groups = [[2 * i, 2 * i + 1] for i in range(n_dev // 2)]
        wv_blk = [pw.tile([P, hc, 512], bf, tag="W", name="wv_blk")
                  for _ in range(hb)]
        qs = max(1, hc // 4)
        for qq in range(0, hc, qs):
            nc.sync.dma_start(wv_blk[0][:, qq:qq + qs, :],
                              d_wv[0, :, qq:qq + qs, :])
            nc.sync.dma_start(scT[:, qq:qq + qs, 0:cpad],
                              d_sc[:, qq:qq + qs, :])
        nc.scalar.mul(zbias[:], scT[:, 0, 0:1], 0.0)
        nc.scalar.copy(warm[:], zbias[:])
        for qq in range(0, hc, qs):
            nc.scalar.dma_start(wv_blk[1][:, qq:qq + qs, :],
                                d_wv[1, :, qq:qq + qs, :])
        nc.sync.dma_start(a0T[:, 0:hc // 2, :], d_a0[:, 0:hc // 2, :])
        nc.sync.dma_start(a0T[:, hc // 2:, :], d_a0[:, hc // 2:, :])
        nc.sync.dma_start(pc_sb[:], d_pc[:])
        # Abs results collect in one contiguous tile; TWO staging DMAs
        # (halves) instead of eight keep the trigger/DMA traffic off the
        # phase-A critical path into the AllGather.
        st_a = psm.tile([P, hv // P, cpad], bf, tag="sa", name="st_a")
        for oi in range(hv // P):
            wt = wv_blk[oi // 4]
            osl = slice((oi % 4) * P, (oi % 4) * P + P)
            for cb in range(cb_n):
                csl = slice(cb * 512, (cb + 1) * 512)
                ps_a = pp.tile([P, 512], f32, tag="PS", name="ps_a")
                for ch in range(hc):
                    nc.tensor.matmul(ps_a[:], wt[:, ch, osl],
                                     scT[:, ch, csl],
                                     start=(ch == 0), stop=(ch == hc - 1))
                nc.scalar.activation(st_a[:, oi, csl], ps_a[:], AF.Abs,
                                     bias=zbias[:])
            if oi % 4 == 3:
                half = oi // 4
                hrows = slice(half * 512, (half + 1) * 512)
                nc.sync.dma_start(
                    agA_in[hrows, :].rearrange("(oc p) c -> p oc c", p=P),
                    st_a[:, half * 4:(half + 1) * 4, :])
        nc.gpsimd.collective_compute(
            "AllGather", mybir.AluOpType.bypass, replica_groups=groups,
            ins=[agA_in.opt()], outs=[agA_out.opt()])

        # ---- Phase K: kMT[r, t] = sum_j MT[j,r] a0T[j,t] -----------------
        # Rows 0-1023 tensor-parallel: own 512 M^T columns -> staging -> a
        # 1MB-in pair AllGather that packs right behind the absc one on
        # the serialized collective resource. Rows 1024-2047 duplicated on
        # both cores (a second 2MB-out collective would not fit the
        # windows; the duplicate costs only 27us of PE).
        agK_in = dram.tile([512, t_trip], bf, name="agK_in", uniquify=False)
        agK_out = dram.tile([2, 512, t_trip], bf, name="agK_out",
                            uniquify=False)

        def emit_k(mt_d, qsplit, chunk0, stage):
            ncols = 512
            mt = pw.tile([P, hc, ncols], bf, tag="W", name="mt_blk")
            for qq in range(0, hc, hc // qsplit):
                nc.scalar.dma_start(mt[:, qq:qq + hc // qsplit, :],
                                    mt_d[:, qq:qq + hc // qsplit, :])
            for oi in range(ncols // P):
                osl = slice(oi * P, (oi + 1) * P)
                st_k = pgs.tile([P, t_trip], bf, tag="SK", name="st_k") \
                    if stage else None
                for tb in range(t_trip // 512):
                    tsl = slice(tb * 512, (tb + 1) * 512)
                    ps_k = pp.tile([P, 512], f32, tag="PS", name="ps_k")
                    for ch in range(hc):
                        nc.tensor.matmul(ps_k[:], mt[:, ch, osl],
                                         a0T[:, ch, tsl],
                                         start=(ch == 0), stop=(ch == hc - 1))
                    if stage:
                        nc.vector.tensor_copy(st_k[:, tsl], ps_k[:])
                    else:
                        nc.vector.tensor_copy(kMT[:, chunk0 + oi, tsl],
                                              ps_k[:])
                if stage:
                    nc.sync.dma_start(agK_in[oi * P:(oi + 1) * P, :],
                                      st_k[:])

        emit_k(d_mtp, 4, 0, stage=True)
        nc.gpsimd.collective_compute(
            "AllGather", mybir.AluOpType.bypass, replica_groups=groups,
            ins=[agK_in.opt()], outs=[agK_out.opt()])
        emit_k(d_mt[0], 2, 8, stage=False)
        # hT streams during phase K's window (its tag-D slot frees once
        # phase A's scT reads finish). On the SP sequencer, before the
        # absc gathers (whose wait would delay it); off the ACT sequencer,
        # where its triggers would delay phase A's Abs ops.
        hT = big.tile([P, hc, s_rows], bf, tag="D", name="hT")
        for qq in range(0, hc, hc // 4):
            nc.sync.dma_start(hT[:, qq:qq + hc // 4, :],
                              d_h[:, qq:qq + hc // 4, :])
        # absc gathers ride the SP ring here: after the kM stagings and hT
        # (which must not wait behind them) and before phase VL's alT
        # loads (which land with slack).
        for g in range(2):
            nc.sync.dma_start(
                abscT[:, g * (hc // 2):(g + 1) * (hc // 2), :],
                agA_out[g].rearrange("(oc p) c -> p oc c", p=P))
        emit_k(d_mt[1], 2, 12, stage=False)

        # Fence: consume the absc gathers once so phase W's instructions
        # decode without unsatisfied waits on the sequencer.
        for fch in (0, hc // 2):
            ps_f = pp.tile([P, 512], f32, tag="PS", name="ps_f")
            nc.tensor.matmul(ps_f[0:1, 0:1], abscT[:, fch, 0:1],
                             dummy[:, 0:1], start=True, stop=True)

        # ---- Phases W+VL, interleaved per own 512-col block --------------
        # W: acWo[c, o] = sum_h abscT[h,c] WoT[h,o]; VL: vo_own[t, o] =
        # lin + scatter. Interleaving (W ob0, VL ob0, W ob1, VL ob1) fires
        # the first vo AllGather ~14us earlier, which shifts the whole
        # serialized collective chain left. Gathered into vo [t, h]
        # (tag A, after a0T).
        agV_in = []
        agV_out = []
        for ob in range(hb):
            agV_in.append(dram.tile([t_trip, 512], bf, name=f"agV_in{ob}",
                                    uniquify=False))
            agV_out.append(dram.tile([2, t_trip, 512], bf,
                                     name=f"agV_out{ob}", uniquify=False))
        vo = big.tile([P, tc_n, h], bf, tag="A", name="vo")
        for ob in range(hb):
            wo = pw.tile([P, hc, 512], bf, tag="W", name="wo_blk")
            nc.scalar.dma_start(wo[:], d_wo[ob])
            for cc in range(cpc):
                ps_w = pp.tile([P, 512], f32, tag="PS", name="ps_w")
                for ch in range(hc):
                    nc.tensor.matmul(ps_w[:], abscT[:, ch, cc * P:cc * P + P],
                                     wo[:, ch, :],
                                     start=(ch == 0), stop=(ch == hc - 1))
                nc.vector.tensor_copy(acWo[:, cc, ob * 512:(ob + 1) * 512],
                                      ps_w[:])
            wvo = pw.tile([P, hc, 512], bf, tag="W", name="wvo_blk")
            nc.scalar.dma_start(wvo[:], d_wvo[ob])
            for tch in range(tc_n):
                al_t = psd.tile([P, hc, P], bf, tag="SD", name="al_t")
                nc.sync.dma_start(al_t[:], d_al[tch])
                tsl = slice(tch * P, (tch + 1) * P)
                ps_v = pp.tile([P, 512], f32, tag="PS", name="ps_v")
                for ch in range(hc):
                    nc.tensor.matmul(ps_v[:], al_t[:, ch, :], wvo[:, ch, :],
                                     start=(ch == 0), stop=False)
                for cc in range(cpc):
                    nc.tensor.matmul(ps_v[:], pc_sb[:, cc, tsl],
                                     acWo[:, cc, ob * 512:(ob + 1) * 512],
                                     start=False, stop=(cc == cpc - 1))
                st_v = pgs.tile([P, 512], bf, tag="ST", name="st_v")
                nc.vector.tensor_copy(st_v[:], ps_v[:])
                # agV stagings ride the ACT ring so the SP ring's alT
                # stream free-runs ahead of the VL groups.
                nc.scalar.dma_start(agV_in[ob][tch * P:(tch + 1) * P, :],
                                    st_v[:])
            nc.gpsimd.collective_compute(
                "AllGather", mybir.AluOpType.bypass, replica_groups=groups,
                ins=[agV_in[ob].opt()], outs=[agV_out[ob].opt()])
        # kM gathers ride the Pool (SWDGE) queue: they wait on the kM
        # AllGather, and on SP/ACT the scheduler interleaves them ahead of
        # later loads, head-of-line blocking the sequencer for the whole
        # wait. Pool's sequencer only hosts the collectives.
        for g in range(2):
            nc.gpsimd.dma_start(
                kMT[:, g * 4:(g + 1) * 4, :],
                agK_out[g].rearrange("(rc p) t -> p rc t", p=P))

        # vo gathers: they wait on the vo AllGathers, so they live on the
        # Pool queue where nothing else needs the sequencer. (Moving the
        # ob1 pair to the faster ACT HWDGE measured 23us WORSE: the
        # scheduler hoists waiting triggers ahead of earlier loads.)
        for ob in range(hb):
            for g in range(2):
                nc.gpsimd.dma_start(
                    vo[:, :, g * hv + ob * 512:g * hv + (ob + 1) * 512],
                    agV_out[ob][g].rearrange("(tc p) c -> p tc c", p=P))

        # ---- Phase S: eT[t, s] = exp(scale * sum_h kMT[h,t] hT[h,s]) -----
        eT = big.tile([P, tc_n, s_rows], bf, tag="F", name="eT")
        ones_t = psm.tile([P, 1], bf, tag="o1", name="ones_t")
        nc.vector.memset(ones_t[:], 1.0)
        ps_sum = ppe.tile([P, 512], f32, tag="PSE", name="ps_sum")
        pse = [ps_sum[32 * st:32 * st + 1, :] for st in range(s512)]

        def emit_ones(tch):
            # exp-sum matmul for chunk tch; deferred one chunk behind the
            # score matmuls so the PE never sits behind the ACT exp.
            for st in range(s512):
                nc.tensor.matmul(pse[st], ones_t[:],
                                 eT[:, tch, st * 512:(st + 1) * 512],
                                 start=(tch == 0), stop=(tch == tc_n - 1))

        for tch in range(tc_n):
            tsl = slice(tch * P, (tch + 1) * P)
            ps_sc = []
            for st in range(s512):
                ps_x = pp.tile([P, 512], f32, tag="PS", name="ps_sc")
                ps_sc.append(ps_x)
                for ch in range(hc):
                    nc.tensor.matmul(ps_x[:], kMT[:, ch, tsl],
                                     hT[:, ch, st * 512:(st + 1) * 512],
                                     start=(ch == 0), stop=(ch == hc - 1))
            if tch > 0:
                emit_ones(tch - 1)
            for st in range(s512):
                nc.scalar.activation(eT[:, tch, st * 512:(st + 1) * 512],
                                     ps_sc[st][:], AF.Exp, bias=zbias[:],
                                     scale=SCALE)
        emit_ones(tc_n - 1)

        # The 1/sum normalization happens on the HOST (out stays
        # unnormalized; the exp-sums ship as a second tiny output): this
        # deletes the recip/transpose/scale machinery from the serial
        # S -> O hinge of the schedule.
        sums_sb = psm.tile([1, s_rows], f32, tag="rc", name="sums_sb")
        for st in range(s512):
            nc.vector.tensor_copy(sums_sb[:, st * 512:(st + 1) * 512],
                                  pse[st])
        nc.sync.dma_start(d_sums[:], sums_sb[:])

        # ---- Phase O: out[s, o] = sum_t eT[t,s] vo[t,o] (unnormalized) ---
        # o-tile order 0,2,1,3: the blocks gathered by the second vo
        # AllGather (global cols 512-1023 and 1536-2047 are ob=1) go last.
        for ot in (0, 2, 1, 3):
            osl = slice(ot * 512, (ot + 1) * 512)
            if ot == 1:
                # Fence: consume the second vo AllGather's gathers with two
                # tiny matmuls so the following groups' instructions
                # decode without unsatisfied waits (the sequencer spends
                # ~0.8us per waiting Ldweights otherwise).
                for fcol in (512, 1536):
                    ps_f = pp.tile([P, 512], f32, tag="PS", name="ps_f")
                    nc.tensor.matmul(ps_f[0:1, 0:1], vo[:, 0, fcol:fcol + 1],
                                     dummy[:, 0:1], start=True, stop=True)
            for sc in range(sc_n):
                ps_o = pp.tile([P, 512], f32, tag="PS", name="ps_o")
                for tch in range(tc_n):
                    nc.tensor.matmul(ps_o[:], eT[:, tch, sc * P:(sc + 1) * P],
                                     vo[:, tch, osl],
                                     start=(tch == 0), stop=(tch == tc_n - 1))
                ob_t = pgs.tile([P, 512], bf, tag="OB", name="ob_t")
                nc.vector.tensor_copy(ob_t[:], ps_o[:])
                # Out-writes ride the SP ring, idle by phase O now that
                # the vo gathers live on the Pool queue; keeping the
                # triggers off the ACT sequencer lets the scale-copies
                # drain back-to-back.
                nc.sync.dma_start(d_out[sc * P:(sc + 1) * P, osl],
                                  ob_t[:])

    nc.compile()
    return nc


def _to_dev_layout(x_t, rows):
    """[rows, n] fp32 -> [128, rows//128, n] bf16 contiguous."""
    rc = rows // P
    return np.ascontiguousarray(
        x_t.reshape(rc, P, -1).transpose(1, 0, 2).astype(bf16))


def _to_chunked_layout(x_t, rows):
    """[rows, n] fp32 -> [n//128, 128, rows//128, 128] bf16 contiguous."""
    dev = _to_dev_layout(x_t, rows)             # [128, rc, n]
    n = dev.shape[2]
    return np.ascontiguousarray(
        dev.reshape(P, rows // P, n // P, P).transpose(2, 0, 1, 3))


def _to_blocked_layout(x_t, rows):
    """[rows, n] fp32 -> [n//512, 128, rows//128, 512] bf16 contiguous.

    512-column blocks of the dev layout, each contiguous in DRAM so a
    streamed [128, hc, 512] weight tile is one dense transfer.
    """
    dev = _to_dev_layout(x_t, rows)             # [128, rc, n]
    n = dev.shape[2]
    return np.ascontiguousarray(
        dev.reshape(P, rows // P, n // 512, 512).transpose(2, 0, 1, 3))


def _gate_prep_merged(trip, rid, cpad):
    """Host-side gate folding: impl and and/or/xor compact rows merged into
    one array (disjoint row sets) and one signed scatter matrix.

    Returns adv_lin [T,h], sc [cpad,h], Pc [T,cpad].
    """
    t_n = trip.shape[0]
    h = trip.shape[2]
    m_and = rid == 0
    m_or = rid == 1
    m_not = rid == 2
    m_impl = rid == 3
    m_xor = rid == 4
    c0 = (rid >= 5).astype(np.float32)
    ca = m_and.astype(np.float32) - m_xor.astype(np.float32)
    cb = m_or.astype(np.float32) + m_xor.astype(np.float32)
    c1 = -(m_not.astype(np.float32))
    ci = m_impl.astype(np.float32)
    k_s = (ca + cb + c1) / 2
    k_d = (c1 - ci) / 2
    k_as = ci / 2
    k_ad = (cb - ca) / 2

    a0 = trip[:, 0]
    asum = trip[:, 1] + trip[:, 2]
    adif = trip[:, 1] - trip[:, 2]
    adv_lin = c0[:, None] * a0 + k_s[:, None] * asum + k_d[:, None] * adif

    impl_idx = np.where(m_impl)[0]
    aox_idx = np.where(m_and | m_or | m_xor)[0]
    n_i, n_a = len(impl_idx), len(aox_idx)
    assert n_i + n_a <= cpad, f"compact rows {n_i + n_a} > pad {cpad}"
    sc = np.zeros((cpad, h), np.float32)
    sc[:n_i] = k_as[impl_idx, None] * asum[impl_idx]
    sc[n_i:n_i + n_a] = np.abs(k_ad[aox_idx, None]) * adif[aox_idx]
    Pc = np.zeros((t_n, cpad), np.float32)
    Pc[impl_idx, np.arange(n_i)] = 1.0
    Pc[aox_idx, n_i + np.arange(n_a)] = np.sign(k_ad[aox_idx])
    return adv_lin, sc, Pc


def kernel(hidden_states, advisor_states, advisor_ids, Wq, Wk, Wv, Wo):
    from concourse.bass_utils import run_bass_kernel_spmd

    hs = np.asarray(hidden_states, dtype=np.float32)     # [4, 2048, 2048]
    adv = np.asarray(advisor_states, dtype=np.float32)   # [4, 3072, 2048]
    ids = np.asarray(advisor_ids)                        # [4, 3072]

    # Size the compact pad to the data (multiple of 512 so the device's
    # 512-wide tiles stay dense). Rebuild only if the data needs more.
    rid_all = ids.reshape(B, T, 3)[:, :, 0]
    need_c = int(max(((rid_all[b] == 0) | (rid_all[b] == 1)
                      | (rid_all[b] == 3) | (rid_all[b] == 4)).sum()
                     for b in range(B)))
    cpad = max(CPAD, -(-need_c // 512) * 512)

    global _compiled_nc
    if _compiled_nc is None or _compiled_nc[0] != cpad:
        _compiled_nc = (cpad, _build_nc4(cpad=cpad))
    nc = _compiled_nc[1]
    Wq = np.asarray(Wq, dtype=np.float32)
    Wk = np.asarray(Wk, dtype=np.float32)
    Wv = np.asarray(Wv, dtype=np.float32)
    Wo = np.asarray(Wo, dtype=np.float32)

    MT = Wk.T @ Wq                          # M^T, M = Wq^T Wk (K-side fold)
    WvT = np.ascontiguousarray(Wv.T)
    WoT = np.ascontiguousarray(Wo.T)
    WVO = WvT @ WoT                         # Wo folded through the v path
    hv = H // 2
    # kM rows 0-1023 are TP'd across the pair (rank g owns cols
    # [g*512, (g+1)*512) of M^T); rows 1024-2047 are duplicated.
    mT_dup = _to_blocked_layout(np.ascontiguousarray(MT[:, hv:]), H)
    w_half = []
    for half in range(2):
        hsl = slice(half * hv, (half + 1) * hv)
        psl = slice(half * 512, (half + 1) * 512)
        w_half.append({
            "wvT": _to_blocked_layout(np.ascontiguousarray(WvT[:, hsl]), H),
            "mTp": _to_dev_layout(np.ascontiguousarray(MT[:, psl]), H),
            "mT": mT_dup,
            "wvoT": _to_blocked_layout(np.ascontiguousarray(WVO[:, hsl]), H),
            "woT": _to_blocked_layout(np.ascontiguousarray(WoT[:, hsl]), H),
        })

    per_batch = []
    for b in range(B):
        trip = adv[b].reshape(T, 3, H)
        rid = ids[b].reshape(T, 3)[:, 0]
        adv_lin, sc, Pc = _gate_prep_merged(trip, rid, cpad)
        per_batch.append({
            "a0T": _to_dev_layout(np.ascontiguousarray(trip[:, 0].T), H),
            "alT": _to_chunked_layout(np.ascontiguousarray(adv_lin.T), H),
            "scT": _to_dev_layout(np.ascontiguousarray(sc.T), H),
            "pcT": _to_dev_layout(np.ascontiguousarray(Pc.T), cpad),
        })

    in_maps = []
    for c in range(NCORES):
        b, half = c // 2, c % 2
        hT = np.ascontiguousarray(hs[b, half * S:(half + 1) * S, :].T)
        m = {
            "hT": _to_dev_layout(hT, H),
            **per_batch[b],
            **w_half[half],
        }
        in_maps.append(m)

    res = run_bass_kernel_spmd(nc, in_maps, core_ids=list(range(NCORES)))
    kernel._last_results = res

    out = np.empty((B, 2 * S, H), dtype=np.float32)
    for c in range(NCORES):
        b, half = c // 2, c % 2
        out[b, half * S:(half + 1) * S, :] = (
            res.results[c]["out"].astype(np.float32)
            / res.results[c]["sums"][0][:, None])
    return out


# revision 78
# speedup vs baseline: 1.0180x; 1.0180x over previous
"""AdvisorCrossAttentionAdapter Trainium2 kernel.

Full inputs in, full outputs out. Sharding: 8 cores = 4 batches x 2 query
halves; the pair sharing a batch also tensor-parallels (by output columns)
the per-batch shared projections, exchanging halves with small pair
AllGathers that hide behind compute.

Math notes (per batch):
  - K projection is folded into the keys: kM = a0 @ M^T with M = Wq^T Wk
    precomputed on the host, so scores = hidden @ kM^T. Folding into K
    (T=1024 rows) instead of Q (S=2048 rows) halves the fold cost.
  - Wo is folded through the value path: with the id-gate rewritten as a
    linear part plus one sparse abs-term (host-compacted, padded to cpad),
        v_final = adv_lin @ Wv^T + Pc @ |sc @ Wv^T|
    the attention output becomes out = attn_norm @ vo, where
        vo = adv_lin @ WVO + Pc @ (|sc @ Wv^T| @ Wo^T),  WVO = Wv^T Wo^T
    (WVO precomputed on the host). The separate ctx and out-projection
    phases collapse into one attn @ vo matmul.
  - Per-core work: abscT 2.15 + kM 6.4 (half TP'd, half duplicated) +
    acWo 2.15 + vo-lin 4.3 + scatter 1.1 + scores 4.3 + out 4.3 =
    24.7 GFLOP (vs 33.3 for the qm-side variant).
  - TP splits are all by OUTPUT COLUMNS (abscT rows = Wv output cols, kM^T
    rows, vo cols), so the pair exchanges are AllGathers (1-2 MB out), not
    the 4 MB AllReduce that sank the earlier vo-partial variant.
  - Softmax runs without max subtraction (scores/sqrt(h) ~ N(0,1)); exp'd
    scores stay unnormalized through the out matmul and the 1/sum factor
    is applied on the HOST (the exp-sums ship as a tiny second output).
  - All matmuls take bf16 inputs with fp32 PSUM accumulation; the output
    is written bf16 and cast to fp32 on the host (~0.2% extra RMS, budget
    is 2e-2).

Schedule notes (the cost model this was tuned against):
  - Collectives serialize on ONE resource at 15us + out_bytes/40GBps each;
    the chain absc -> kM -> vo0 -> vo1 (4 x 67.4us, back to back from
    ~46us) IS the critical path, so: the absc AllGather launches straight
    off phase A; kM is only half-TP'd (the other half's duplicated compute
    is cheaper than a bigger collective and covers the absc window); the
    acWo and vo phases interleave per 512-col block so the first vo
    AllGather fires mid-phase; phases S and the first half of O run under
    the vo collectives; O's o-tiles run in order 0,2,1,3 so only its last
    two tiles wait on the second vo AllGather.
  - All DMA shares ONE serial ~358GB/s resource; a DMA trigger occupies
    its engine's sequencer until dispatched, and the bass scheduler may
    reorder same-engine triggers, so collective-dependent gathers live on
    the otherwise-idle Pool (SWDGE) queue, input streams split between the
    SP/ACT rings in consumption order, and phase A's stationary+rhs
    interleave finely at the front.
"""

import numpy as np
import ml_dtypes
from contextlib import ExitStack

P = 128
H = 2048          # hidden dim
HC = H // P       # 16 h-chunks of 128
T = 1024          # triplets per batch (advisor len 3072 / 3)
TC = T // P       # 8 t-chunks
S = 1024          # query rows per core (2048 / 2)
B = 4
NCORES = 8
CPAD = 512        # padded compact rows (impl + and/or/xor, disjoint)
SCALE = 1.0 / float(np.sqrt(H))

bf16 = ml_dtypes.bfloat16

_compiled_nc = None


def _build_nc4(s_rows=S, t_trip=T, h=H, cpad=CPAD, n_dev=NCORES,
               stop_after=None):
    import concourse.bass as bass
    import concourse.mybir as mybir
    import concourse.tile as tile
    from concourse import bacc

    hc = h // P          # 16 h-chunks
    tc_n = t_trip // P   # 8 t-chunks
    sc_n = s_rows // P   # 8 s-chunks
    s512 = s_rows // 512 # 2
    hv = h // 2          # own half of output columns
    hb = hv // 512       # 2 own 512-blocks
    cpc = cpad // P      # compact-row chunks
    cb_n = cpad // 512   # compact 512-tiles
    n512 = h // 512      # 4 full-width o tiles (phase O)
    assert s_rows % 512 == 0 and h % 1024 == 0 and cpad % 512 == 0

    f32 = mybir.dt.float32
    bf = mybir.dt.bfloat16

    nc = bacc.Bacc("TRN2", target_bir_lowering=False, debug=False,
                   num_devices=n_dev)

    # DRAM I/O. All weight streams are pre-blocked on the host into
    # [block, 128, hc, 512] so each streamed tile is contiguous per
    # partition. "own" = this core's half of the respective output columns
    # (selected purely by the data the host feeds it; the program is SPMD).
    d_sc = nc.dram_tensor("scT", [P, hc, cpad], bf, kind="ExternalInput")
    d_wv = nc.dram_tensor("wvT", [hb, P, hc, 512], bf, kind="ExternalInput")
    # kM rows: the first 1024 (2 x 512-blocks) are tensor-parallel across
    # the pair (mTp = own 512 columns of M^T), the last 1024 duplicated.
    # (TP'ing fewer rows shrinks the kM AllGather but grows the duplicated
    # K compute, which delays phase VL and the vo AllGathers: measured
    # worse.)
    d_mtp = nc.dram_tensor("mTp", [P, hc, 512], bf, kind="ExternalInput")
    d_mt = nc.dram_tensor("mT", [2, P, hc, 512], bf, kind="ExternalInput")
    d_a0 = nc.dram_tensor("a0T", [P, hc, t_trip], bf, kind="ExternalInput")
    d_al = nc.dram_tensor("alT", [t_trip // P, P, hc, P], bf,
                          kind="ExternalInput")
    d_pc = nc.dram_tensor("pcT", [P, cpc, t_trip], bf, kind="ExternalInput")
    d_wvo = nc.dram_tensor("wvoT", [hb, P, hc, 512], bf,
                           kind="ExternalInput")
    d_wo = nc.dram_tensor("woT", [hb, P, hc, 512], bf, kind="ExternalInput")
    d_h = nc.dram_tensor("hT", [P, hc, s_rows], bf, kind="ExternalInput")
    d_out = nc.dram_tensor("out", [s_rows, h], bf,
                            kind="ExternalOutput")
    d_sums = nc.dram_tensor("sums", [1, s_rows], f32,
                            kind="ExternalOutput")

    AF = mybir.ActivationFunctionType

    with tile.TileContext(nc) as tc, ExitStack() as ctx:
        big = ctx.enter_context(tc.tile_pool(name="big", bufs=1))
        pw = ctx.enter_context(tc.tile_pool(name="pw", bufs=2))
        psd = ctx.enter_context(tc.tile_pool(name="psd", bufs=2))
        pgs = ctx.enter_context(tc.tile_pool(name="pgs", bufs=2))
        psm = ctx.enter_context(tc.tile_pool(name="psm", bufs=1))
        pp = ctx.enter_context(tc.tile_pool(name="pp", bufs=6, space="PSUM"))
        ppe = ctx.enter_context(tc.tile_pool(name="ppe", bufs=1,
                                             space="PSUM"))
        dram = ctx.enter_context(tc.tile_pool(name="dram", bufs=1,
                                              space="DRAM"))

        # Persistent intermediates. Tag reuse: a0T -> vo (phase K ends
        # before the vo gather lands), scT -> hT (phase A ends before the
        # hT stream is issued on the weight queue).
        a0T = big.tile([P, hc, t_trip], bf, tag="A", name="a0T")
        kMT = big.tile([P, hc, t_trip], bf, tag="C", name="kMT")
        scT = big.tile([P, hc, max(cpad, 1024)], bf, tag="D", name="scT")
        abscT = big.tile([P, hc, cpad], bf, tag="E", name="abscT")
        pc_sb = psm.tile([P, cpc, t_trip], bf, tag="pc", name="pc_sb")
        acWo = psm.tile([P, cpc, hv], bf, tag="aw", name="acWo")

        # ACT-written zero bias vector so Abs/Exp activations don't pull in
        # a DMA'd const AP (also absorbs the pipeline-RAW wait). Sourced
        # from scT, the first tile to land on the sync queue.
        zbias = psm.tile([P, 1], f32, tag="zb", name="zbias")
        warm = psm.tile([P, 1], f32, tag="wm", name="warm")

        # PE warm-up while the first weight tiles land.
        dummy = psm.tile([P, 512], bf, tag="dm", name="dummy")
        nc.vector.memset(dummy[:], 0.0)
        for _ in range(10):
            ps_dm = pp.tile([P, 512], f32, tag="PS", name="ps_dm")
            nc.tensor.matmul(ps_dm[:], dummy[:, 0:P], dummy[:],
                             start=True, stop=True)

        # Input streams. The sim models ONE serial DMA resource, so global
        # order matters: phase A's stationary (wv block 0) and rhs (scT)
        # stream first, finely interleaved so the first groups ramp with
        # the DMA; everything phase A doesn't need follows on the ACT ring.

        # ---- Phase A: abscT_own[o, c] = |sum_j WvT[j,o] scT[j,c]| --------
        # o = own half of Wv output columns (8 chunks). Staged to DRAM and
        # pair-AllGathered into the full abscT [h, cpad]. (A 2-way split
        # of this collective measured WORSE: ~7us of event/SEQ latency per
        # collective eats the earlier launch.)
        agA_in = dram.tile([hv, cpad], bf, name="agA_in", uniquify=False)
        agA_out = dram.tile([2, hv, cpad], bf, name="agA_out",
                            uniquify=False)
        groups = [[2 * i, 2 * i + 1] for i in range(n_dev // 2)]
        wv_blk = [pw.tile([P, hc, 512], bf, tag="W", name="wv_blk")
                  for _ in range(hb)]
        qs = max(1, hc // 4)
        for qq in range(0, hc, qs):
            nc.sync.dma_start(wv_blk[0][:, qq:qq + qs, :],
                              d_wv[0, :, qq:qq + qs, :])
            nc.sync.dma_start(scT[:, qq:qq + qs, 0:cpad],
                              d_sc[:, qq:qq + qs, :])
        nc.scalar.mul(zbias[:], scT[:, 0, 0:1], 0.0)
        nc.scalar.copy(warm[:], zbias[:])
        for qq in range(0, hc, qs):
            nc.scalar.dma_start(wv_blk[1][:, qq:qq + qs, :],
                                d_wv[1, :, qq:qq + qs, :])
        nc.sync.dma_start(a0T[:, 0:hc // 2, :], d_a0[:, 0:hc // 2, :])
        nc.sync.dma_start(a0T[:, hc // 2:, :], d_a0[:, hc // 2:, :])
        nc.sync.dma_start(pc_sb[:], d_pc[:])
        # Abs results collect in one contiguous tile; TWO staging DMAs
        # (halves) instead of eight keep the trigger/DMA traffic off the
        # phase-A critical path into the AllGather.
        st_a = psm.tile([P, hv // P, cpad], bf, tag="sa", name="st_a")
        for oi in range(hv // P):
            wt = wv_blk[oi // 4]
            osl = slice((oi % 4) * P, (oi % 4) * P + P)
            for cb in range(cb_n):
                csl = slice(cb * 512, (cb + 1) * 512)
                ps_a = pp.tile([P, 512], f32, tag="PS", name="ps_a")
                for ch in range(hc):
                    nc.tensor.matmul(ps_a[:], wt[:, ch, osl],
                                     scT[:, ch, csl],
                                     start=(ch == 0), stop=(ch == hc - 1))
                nc.scalar.activation(st_a[:, oi, csl], ps_a[:], AF.Abs,
                                     bias=zbias[:])
            if oi % 4 == 3:
                half = oi // 4
                hrows = slice(half * 512, (half + 1) * 512)
                nc.sync.dma_start(
                    agA_in[hrows, :].rearrange("(oc p) c -> p oc c", p=P),
                    st_a[:, half * 4:(half + 1) * 4, :])
        nc.gpsimd.collective_compute(
            "AllGather", mybir.AluOpType.bypass, replica_groups=groups,
            ins=[agA_in.opt()], outs=[agA_out.opt()])

        # ---- Phase K: kMT[r, t] = sum_j MT[j,r] a0T[j,t] -----------------
        # Rows 0-1023 tensor-parallel: own 512 M^T columns -> staging -> a
        # 1MB-in pair AllGather that packs right behind the absc one on
        # the serialized collective resource. Rows 1024-2047 duplicated on
        # both cores (a second 2MB-out collective would not fit the
        # windows; the duplicate costs only 27us of PE).
        agK_in = dram.tile([512, t_trip], bf, name="agK_in", uniquify=False)
        agK_out = dram.tile([2, 512, t_trip], bf, name="agK_out",
                            uniquify=False)

        def emit_k(mt_d, qsplit, chunk0, stage, eng=None):
            ncols = 512
            mt = pw.tile([P, hc, ncols], bf, tag="W", name="mt_blk")
            for qq in range(0, hc, hc // qsplit):
                (eng or nc.scalar).dma_start(mt[:, qq:qq + hc // qsplit, :],
                                             mt_d[:, qq:qq + hc // qsplit, :])
            for oi in range(ncols // P):
                osl = slice(oi * P, (oi + 1) * P)
                st_k = pgs.tile([P, t_trip], bf, tag="SK", name="st_k") \
                    if stage else None
                for tb in range(t_trip // 512):
                    tsl = slice(tb * 512, (tb + 1) * 512)
                    ps_k = pp.tile([P, 512], f32, tag="PS", name="ps_k")
                    for ch in range(hc):
                        nc.tensor.matmul(ps_k[:], mt[:, ch, osl],
                                         a0T[:, ch, tsl],
                                         start=(ch == 0), stop=(ch == hc - 1))
                    if stage:
                        nc.vector.tensor_copy(st_k[:, tsl], ps_k[:])
                    else:
                        nc.vector.tensor_copy(kMT[:, chunk0 + oi, tsl],
                                              ps_k[:])
                if stage:
                    nc.sync.dma_start(agK_in[oi * P:(oi + 1) * P, :],
                                      st_k[:])

        emit_k(d_mtp, 4, 0, stage=True, eng=nc.sync)
        nc.gpsimd.collective_compute(
            "AllGather", mybir.AluOpType.bypass, replica_groups=groups,
            ins=[agK_in.opt()], outs=[agK_out.opt()])
        emit_k(d_mt[0], 2, 8, stage=False, eng=nc.sync)
        # hT streams during phase K's window (its tag-D slot frees once
        # phase A's scT reads finish). On the SP sequencer, before the
        # absc gathers (whose wait would delay it); off the ACT sequencer,
        # where its triggers would delay phase A's Abs ops.
        hT = big.tile([P, hc, s_rows], bf, tag="D", name="hT")
        for qq in range(0, hc, hc // 4):
            nc.sync.dma_start(hT[:, qq:qq + hc // 4, :],
                              d_h[:, qq:qq + hc // 4, :])
        # absc gathers ride the SP ring here: after the kM stagings and hT
        # (which must not wait behind them) and before phase VL's alT
        # loads (which land with slack).
        for g in range(2):
            nc.sync.dma_start(
                abscT[:, g * (hc // 2):(g + 1) * (hc // 2), :],
                agA_out[g].rearrange("(oc p) c -> p oc c", p=P))
        emit_k(d_mt[1], 2, 12, stage=False)

        # Fence: consume the absc gathers once so phase W's instructions
        # decode without unsatisfied waits on the sequencer.
        for fch in (0, hc // 2):
            ps_f = pp.tile([P, 512], f32, tag="PS", name="ps_f")
            nc.tensor.matmul(ps_f[0:1, 0:1], abscT[:, fch, 0:1],
                             dummy[:, 0:1], start=True, stop=True)

        # ---- Phases W+VL, interleaved per own 512-col block --------------
        # W: acWo[c, o] = sum_h abscT[h,c] WoT[h,o]; VL: vo_own[t, o] =
        # lin + scatter. Interleaving (W ob0, VL ob0, W ob1, VL ob1) fires
        # the first vo AllGather ~14us earlier, which shifts the whole
        # serialized collective chain left. Gathered into vo [t, h]
        # (tag A, after a0T).
        agV_in = []
        agV_out = []
        for ob in range(hb):
            agV_in.append(dram.tile([t_trip, 512], bf, name=f"agV_in{ob}",
                                    uniquify=False))
            agV_out.append(dram.tile([2, t_trip, 512], bf,
                                     name=f"agV_out{ob}", uniquify=False))
        vo = big.tile([P, tc_n, h], bf, tag="A", name="vo")
        for ob in range(hb):
            wo = pw.tile([P, hc, 512], bf, tag="W", name="wo_blk")
            nc.scalar.dma_start(wo[:], d_wo[ob])
            for cc in range(cpc):
                ps_w = pp.tile([P, 512], f32, tag="PS", name="ps_w")
                for ch in range(hc):
                    nc.tensor.matmul(ps_w[:], abscT[:, ch, cc * P:cc * P + P],
                                     wo[:, ch, :],
                                     start=(ch == 0), stop=(ch == hc - 1))
                nc.vector.tensor_copy(acWo[:, cc, ob * 512:(ob + 1) * 512],
                                      ps_w[:])
            wvo = pw.tile([P, hc, 512], bf, tag="W", name="wvo_blk")
            nc.scalar.dma_start(wvo[:], d_wvo[ob])
            for tch in range(tc_n):
                al_t = psd.tile([P, hc, P], bf, tag="SD", name="al_t")
                nc.sync.dma_start(al_t[:], d_al[tch])
                tsl = slice(tch * P, (tch + 1) * P)
                ps_v = pp.tile([P, 512], f32, tag="PS", name="ps_v")
                for ch in range(hc):
                    nc.tensor.matmul(ps_v[:], al_t[:, ch, :], wvo[:, ch, :],
                                     start=(ch == 0), stop=False)
                for cc in range(cpc):
                    nc.tensor.matmul(ps_v[:], pc_sb[:, cc, tsl],
                                     acWo[:, cc, ob * 512:(ob + 1) * 512],
                                     start=False, stop=(cc == cpc - 1))
                st_v = pgs.tile([P, 512], bf, tag="ST", name="st_v")
                nc.vector.tensor_copy(st_v[:], ps_v[:])
                # agV stagings ride the ACT ring so the SP ring's alT
                # stream free-runs ahead of the VL groups.
                nc.scalar.dma_start(agV_in[ob][tch * P:(tch + 1) * P, :],
                                    st_v[:])
            nc.gpsimd.collective_compute(
                "AllGather", mybir.AluOpType.bypass, replica_groups=groups,
                ins=[agV_in[ob].opt()], outs=[agV_out[ob].opt()])
        # kM gathers ride the Pool (SWDGE) queue: they wait on the kM
        # AllGather, and on SP/ACT the scheduler interleaves them ahead of
        # later loads, head-of-line blocking the sequencer for the whole
        # wait. Pool's sequencer only hosts the collectives.
        for g in range(2):
            nc.gpsimd.dma_start(
                kMT[:, g * 4:(g + 1) * 4, :],
                agK_out[g].rearrange("(rc p) t -> p rc t", p=P))

        # vo gathers: they wait on the vo AllGathers, so they live on the
        # Pool queue where nothing else needs the sequencer. (Moving the
        # ob1 pair to the faster ACT HWDGE measured 23us WORSE: the
        # scheduler hoists waiting triggers ahead of earlier loads.)
        for ob in range(hb):
            for g in range(2):
                nc.gpsimd.dma_start(
                    vo[:, :, g * hv + ob * 512:g * hv + (ob + 1) * 512],
                    agV_out[ob][g].rearrange("(tc p) c -> p tc c", p=P))

        # ---- Phase S: eT[t, s] = exp(scale * sum_h kMT[h,t] hT[h,s]) -----
        eT = big.tile([P, tc_n, s_rows], bf, tag="F", name="eT")
        ones_t = psm.tile([P, 1], bf, tag="o1", name="ones_t")
        nc.vector.memset(ones_t[:], 1.0)
        ps_sum = ppe.tile([P, 512], f32, tag="PSE", name="ps_sum")
        pse = [ps_sum[32 * st:32 * st + 1, :] for st in range(s512)]

        def emit_ones(tch):
            # exp-sum matmul for chunk tch; deferred one chunk behind the
            # score matmuls so the PE never sits behind the ACT exp.
            for st in range(s512):
                nc.tensor.matmul(pse[st], ones_t[:],
                                 eT[:, tch, st * 512:(st + 1) * 512],
                                 start=(tch == 0), stop=(tch == tc_n - 1))

        for tch in range(tc_n):
            tsl = slice(tch * P, (tch + 1) * P)
            ps_sc = []
            for st in range(s512):
                ps_x = pp.tile([P, 512], f32, tag="PS", name="ps_sc")
                ps_sc.append(ps_x)
                for ch in range(hc):
                    nc.tensor.matmul(ps_x[:], kMT[:, ch, tsl],
                                     hT[:, ch, st * 512:(st + 1) * 512],
                                     start=(ch == 0), stop=(ch == hc - 1))
            if tch > 0:
                emit_ones(tch - 1)
            for st in range(s512):
                nc.scalar.activation(eT[:, tch, st * 512:(st + 1) * 512],
                                     ps_sc[st][:], AF.Exp, bias=zbias[:],
                                     scale=SCALE)
        emit_ones(tc_n - 1)

        # The 1/sum normalization happens on the HOST (out stays
        # unnormalized; the exp-sums ship as a second tiny output): this
        # deletes the recip/transpose/scale machinery from the serial
        # S -> O hinge of the schedule.
        sums_sb = psm.tile([1, s_rows], f32, tag="rc", name="sums_sb")
        for st in range(s512):
            nc.vector.tensor_copy(sums_sb[:, st * 512:(st + 1) * 512],
                                  pse[st])
        nc.sync.dma_start(d_sums[:], sums_sb[:])

        # ---- Phase O: out[s, o] = sum_t eT[t,s] vo[t,o] (unnormalized) ---
        # o-tile order 0,2,1,3: the blocks gathered by the second vo
        # AllGather (global cols 512-1023 and 1536-2047 are ob=1) go last.
        for ot in (0, 2, 1, 3):
            osl = slice(ot * 512, (ot + 1) * 512)
            if ot == 1:
                # Fence: consume the second vo AllGather's gathers with two
                # tiny matmuls so the following groups' instructions
                # decode without unsatisfied waits (the sequencer spends
                # ~0.8us per waiting Ldweights otherwise).
                for fcol in (512, 1536):
                    ps_f = pp.tile([P, 512], f32, tag="PS", name="ps_f")
                    nc.tensor.matmul(ps_f[0:1, 0:1], vo[:, 0, fcol:fcol + 1],
                                     dummy[:, 0:1], start=True, stop=True)
            for sc in range(sc_n):
                ps_o = pp.tile([P, 512], f32, tag="PS", name="ps_o")
                for tch in range(tc_n):
                    nc.tensor.matmul(ps_o[:], eT[:, tch, sc * P:(sc + 1) * P],
                                     vo[:, tch, osl],
                                     start=(tch == 0), stop=(tch == tc_n - 1))
                ob_t = pgs.tile([P, 512], bf, tag="OB", name="ob_t")
                nc.vector.tensor_copy(ob_t[:], ps_o[:])
                # Out-writes ride the SP ring, idle by phase O now that
                # the vo gathers live on the Pool queue; keeping the
                # triggers off the ACT sequencer lets the scale-copies
                # drain back-to-back.
                nc.sync.dma_start(d_out[sc * P:(sc + 1) * P, osl],
                                  ob_t[:])

    nc.compile()
    return nc


def _to_dev_layout(x_t, rows):
    """[rows, n] fp32 -> [128, rows//128, n] bf16 contiguous."""
    rc = rows // P
    return np.ascontiguousarray(
        x_t.reshape(rc, P, -1).transpose(1, 0, 2).astype(bf16))


def _to_chunked_layout(x_t, rows):
    """[rows, n] fp32 -> [n//128, 128, rows//128, 128] bf16 contiguous."""
    dev = _to_dev_layout(x_t, rows)             # [128, rc, n]
    n = dev.shape[2]
    return np.ascontiguousarray(
        dev.reshape(P, rows // P, n // P, P).transpose(2, 0, 1, 3))


def _to_blocked_layout(x_t, rows):
    """[rows, n] fp32 -> [n//512, 128, rows//128, 512] bf16 contiguous.

    512-column blocks of the dev layout, each contiguous in DRAM so a
    streamed [128, hc, 512] weight tile is one dense transfer.
    """
    dev = _to_dev_layout(x_t, rows)             # [128, rc, n]
    n = dev.shape[2]
    return np.ascontiguousarray(
        dev.reshape(P, rows // P, n // 512, 512).transpose(2, 0, 1, 3))


def _gate_prep_merged(trip, rid, cpad):
    """Host-side gate folding: impl and and/or/xor compact rows merged into
    one array (disjoint row sets) and one signed scatter matrix.

    Returns adv_lin [T,h], sc [cpad,h], Pc [T,cpad].
    """
    t_n = trip.shape[0]
    h = trip.shape[2]
    m_and = rid == 0
    m_or = rid == 1
    m_not = rid == 2
    m_impl = rid == 3
    m_xor = rid == 4
    c0 = (rid >= 5).astype(np.float32)
    ca = m_and.astype(np.float32) - m_xor.astype(np.float32)
    cb = m_or.astype(np.float32) + m_xor.astype(np.float32)
    c1 = -(m_not.astype(np.float32))
    ci = m_impl.astype(np.float32)
    k_s = (ca + cb + c1) / 2
    k_d = (c1 - ci) / 2
    k_as = ci / 2
    k_ad = (cb - ca) / 2

    a0 = trip[:, 0]
    asum = trip[:, 1] + trip[:, 2]
    adif = trip[:, 1] - trip[:, 2]
    adv_lin = c0[:, None] * a0 + k_s[:, None] * asum + k_d[:, None] * adif

    impl_idx = np.where(m_impl)[0]
    aox_idx = np.where(m_and | m_or | m_xor)[0]
    n_i, n_a = len(impl_idx), len(aox_idx)
    assert n_i + n_a <= cpad, f"compact rows {n_i + n_a} > pad {cpad}"
    sc = np.zeros((cpad, h), np.float32)
    sc[:n_i] = k_as[impl_idx, None] * asum[impl_idx]
    sc[n_i:n_i + n_a] = np.abs(k_ad[aox_idx, None]) * adif[aox_idx]
    Pc = np.zeros((t_n, cpad), np.float32)
    Pc[impl_idx, np.arange(n_i)] = 1.0
    Pc[aox_idx, n_i + np.arange(n_a)] = np.sign(k_ad[aox_idx])
    return adv_lin, sc, Pc


def kernel(hidden_states, advisor_states, advisor_ids, Wq, Wk, Wv, Wo):
    from concourse.bass_utils import run_bass_kernel_spmd

    hs = np.asarray(hidden_states, dtype=np.float32)     # [4, 2048, 2048]
    adv = np.asarray(advisor_states, dtype=np.float32)   # [4, 3072, 2048]
    ids = np.asarray(advisor_ids)                        # [4, 3072]

    # Size the compact pad to the data (multiple of 512 so the device's
    # 512-wide tiles stay dense). Rebuild only if the data needs more.
    rid_all = ids.reshape(B, T, 3)[:, :, 0]
    need_c = int(max(((rid_all[b] == 0) | (rid_all[b] == 1)
                      | (rid_all[b] == 3) | (rid_all[b] == 4)).sum()
                     for b in range(B)))
    cpad = max(CPAD, -(-need_c // 512) * 512)

    global _compiled_nc
    if _compiled_nc is None or _compiled_nc[0] != cpad:
        _compiled_nc = (cpad, _build_nc4(cpad=cpad))
    nc = _compiled_nc[1]
    Wq = np.asarray(Wq, dtype=np.float32)
    Wk = np.asarray(Wk, dtype=np.float32)
    Wv = np.asarray(Wv, dtype=np.float32)
    Wo = np.asarray(Wo, dtype=np.float32)

    MT = Wk.T @ Wq                          # M^T, M = Wq^T Wk (K-side fold)
    WvT = np.ascontiguousarray(Wv.T)
    WoT = np.ascontiguousarray(Wo.T)
    WVO = WvT @ WoT                         # Wo folded through the v path
    hv = H // 2
    # kM rows 0-1023 are TP'd across the pair (rank g owns cols
    # [g*512, (g+1)*512) of M^T); rows 1024-2047 are duplicated.
    mT_dup = _to_blocked_layout(np.ascontiguousarray(MT[:, hv:]), H)
    w_half = []
    for half in range(2):
        hsl = slice(half * hv, (half + 1) * hv)
        psl = slice(half * 512, (half + 1) * 512)
        w_half.append({
            "wvT": _to_blocked_layout(np.ascontiguousarray(WvT[:, hsl]), H),
            "mTp": _to_dev_layout(np.ascontiguousarray(MT[:, psl]), H),
            "mT": mT_dup,
            "wvoT": _to_blocked_layout(np.ascontiguousarray(WVO[:, hsl]), H),
            "woT": _to_blocked_layout(np.ascontiguousarray(WoT[:, hsl]), H),
        })

    per_batch = []
    for b in range(B):
        trip = adv[b].reshape(T, 3, H)
        rid = ids[b].reshape(T, 3)[:, 0]
        adv_lin, sc, Pc = _gate_prep_merged(trip, rid, cpad)
        per_batch.append({
            "a0T": _to_dev_layout(np.ascontiguousarray(trip[:, 0].T), H),
            "alT": _to_chunked_layout(np.ascontiguousarray(adv_lin.T), H),
            "scT": _to_dev_layout(np.ascontiguousarray(sc.T), H),
            "pcT": _to_dev_layout(np.ascontiguousarray(Pc.T), cpad),
        })

    in_maps = []
    for c in range(NCORES):
        b, half = c // 2, c % 2
        hT = np.ascontiguousarray(hs[b, half * S:(half + 1) * S, :].T)
        m = {
            "hT": _to_dev_layout(hT, H),
            **per_batch[b],
            **w_half[half],
        }
        in_maps.append(m)

    res = run_bass_kernel_spmd(nc, in_maps, core_ids=list(range(NCORES)))
    kernel._last_results = res

    out = np.empty((B, 2 * S, H), dtype=np.float32)
    for c in range(NCORES):
        b, half = c // 2, c % 2
        out[b, half * S:(half + 1) * S, :] = (
            res.results[c]["out"].astype(np.float32)
            / res.results[c]["sums"][0][:, None])
    return out
